# revision 1
# baseline (speedup 1.0000x reference)
"""Trainium2 Bass kernel for a bidirectional-LSTM language model.

Model (see problem reference): x = emb[tokens]; h = concat(LSTM_fwd(x),
LSTM_bwd(x)); out = softmax(h @ Wd + bd).  V=32000, E=256, H=512, T=127, B=16.

Sharding: one uniform SPMD program on 8 cores. Core 0 computes the forward
LSTM, core 1 the backward LSTM (fed host-time-reversed tokens); per-core
{0,1}-masks select whose hidden states enter an AllReduce that broadcasts
h^T to everyone. The vocab dimension of the Dense+softmax is sharded 8-way
(4000 per core); softmax denominators are combined with a second tiny
AllReduce. Each core writes its own [B,T,4000] fp32 slice; the host
concatenates.

Layouts: everything runs transposed ([feature, token]) so LSTM gate math and
dense stationary operands sit on 128 partitions. Token order is time-major
(col = t*B + b). The LSTM recurrent matmul keeps bf16 R tiles stationary
(fast weight load) and streams the 16 batch columns, 64 matmuls per step;
the gate-dim m-tiles are permuted to [g,i,f,o] so the elementwise tail after
the last matmul is short.
"""

import numpy as np
import ml_dtypes

import concourse.bass as bass
import concourse.mybir as mybir
import concourse.tile as tile
from concourse import bacc
from concourse.bass import ts, ds
from concourse.bass_utils import run_bass_kernel_spmd
from concourse.masks import make_identity

F32 = mybir.dt.float32
BF16 = mybir.dt.bfloat16
FP16 = mybir.dt.float16
I32 = mybir.dt.int32
AF = mybir.ActivationFunctionType
ALU = mybir.AluOpType

V, E, H, T, B = 32000, 256, 512, 127, 16
G4 = 4 * H              # 2048
NTOK = T * B            # 2032
NCORES = 8
VC = V // NCORES        # 4000 vocab per core
VCH = 500               # vocab chunk (<=512 psum free dim)
NVCH = VC // VCH        # 8
NKD = 2 * H // 128      # 8 k-tiles for dense
NTT = (NTOK + 127) // 128   # 16 token tiles (last = 112)
# m-tile semantic order for the 2048-wide gate dim: [g, i, f, o]
PERM = [8, 9, 10, 11, 0, 1, 2, 3, 4, 5, 6, 7, 12, 13, 14, 15]
GATE_G, GATE_I, GATE_F, GATE_O = 0, 1, 2, 3   # gate block index in permuted order

_BUILT = None


def build_kernel(n_steps=T):
    nc = bacc.Bacc("TRN2", target_bir_lowering=False, debug=False,
                   num_devices=NCORES)

    tok = nc.dram_tensor("tok", [NTOK], I32, kind="ExternalInput")
    emb = nc.dram_tensor("emb", [V, E], F32, kind="ExternalInput")
    kmat = nc.dram_tensor("kmat", [E, G4], F32, kind="ExternalInput")
    rmat = nc.dram_tensor("rmat", [H, G4], F32, kind="ExternalInput")
    bvec = nc.dram_tensor("bvec", [G4], F32, kind="ExternalInput")
    wd = nc.dram_tensor("wd", [2 * H, VC], BF16, kind="ExternalInput")
    bd = nc.dram_tensor("bd", [1, VC], BF16, kind="ExternalInput")
    maskf = nc.dram_tensor("maskf", [128, 1], F32, kind="ExternalInput")
    maskb = nc.dram_tensor("maskb", [128, 1], F32, kind="ExternalInput")
    out = nc.dram_tensor("out", [B, T, VC], F32, kind="ExternalOutput")

    with tile.TileContext(nc) as tc:
        with (
            tc.tile_pool(name="persist", bufs=1) as persist,
            tc.tile_pool(name="dram", bufs=1, space="DRAM") as dram,
        ):
            ident = persist.tile([128, 128], F32)
            make_identity(nc, ident[:])
            mf_t = persist.tile([128, 1], F32, tag="mf")
            nc.gpsimd.dma_start(mf_t[:], maskf[:])
            mb_t = persist.tile([128, 1], F32, tag="mb")
            nc.gpsimd.dma_start(mb_t[:], maskb[:])

            with tc.tile_pool(name="abpool", bufs=1) as abpool:
                preT = abpool.tile([128, 16, NTOK], BF16)      # [g,i,f,o] m-order
                hT = abpool.tile([128, 4, T + 1, B], BF16)     # h^T, col0 = h_0 = 0

                # ---- Phase A: embed gather, x^T, pre^T = k^T x^T + b ----
                with (
                    tc.tile_pool(name="apool", bufs=1) as apool,
                    tc.tile_pool(name="aio", bufs=3) as aio,
                    tc.tile_pool(name="apsum", bufs=3, space="PSUM") as apsum,
                ):
                    toki = apool.tile([128, NTT], I32)
                    nc.gpsimd.dma_start(
                        toki[:, :NTT - 1],
                        tok[:(NTT - 1) * 128].rearrange("(n p) -> p n", p=128))
                    nc.gpsimd.dma_start(
                        toki[:112, NTT - 1:NTT],
                        tok[ds((NTT - 1) * 128, 112)].rearrange("(n p) -> p n", p=112))

                    bcol = apool.tile([128, 16], F32)
                    for m in range(16):
                        nc.gpsimd.dma_start(
                            bcol[:, m:m + 1],
                            bvec[ts(PERM[m], 128)].rearrange("(n p) -> p n", p=128))

                    km_f = apool.tile([128, 2, G4], F32)
                    nc.gpsimd.dma_start(km_f[:],
                                        kmat.rearrange("(a p) g -> p a g", p=128))
                    km_b = apool.tile([128, 2, G4], BF16)
                    nc.vector.tensor_copy(km_b[:], km_f[:])

                    xT = apool.tile([128, 2, NTOK], BF16)
                    for j in range(NTT):
                        rows = 128 if j < NTT - 1 else NTOK - 128 * (NTT - 1)
                        xg = aio.tile([128, E], F32, tag="xg")
                        nc.gpsimd.indirect_dma_start(
                            out=xg[:rows, :], out_offset=None, in_=emb[:, :],
                            in_offset=bass.IndirectOffsetOnAxis(
                                ap=toki[:rows, j:j + 1], axis=0),
                        )
                        for e in range(2):
                            pst = apsum.tile([128, 128], F32, tag="pst")
                            nc.tensor.transpose(pst[:, :rows],
                                                xg[:rows, ts(e, 128)],
                                                ident[:rows, :rows])
                            nc.vector.tensor_copy(xT[:, e, ds(128 * j, rows)],
                                                  pst[:, :rows])

                    for m in range(16):
                        for nch in range(4):
                            ppre = apsum.tile([128, 508], F32, tag="ppre")
                            for k in range(2):
                                nc.tensor.matmul(
                                    ppre[:], km_b[:, k, ts(PERM[m], 128)],
                                    xT[:, k, ds(nch * 508, 508)],
                                    start=(k == 0), stop=(k == 1))
                            nc.scalar.activation(
                                preT[:, m, ds(nch * 508, 508)], ppre[:],
                                AF.Identity, bias=bcol[:, m:m + 1], scale=1.0)

                # ---- Phase B: LSTM over time ----
                with (
                    tc.tile_pool(name="bpool", bufs=1) as bpool,
                    tc.tile_pool(name="bstage", bufs=2) as bstage,
                    tc.tile_pool(name="zpool", bufs=2, space="PSUM") as zpool,
                    tc.tile_pool(name="gwork", bufs=3) as gwork,
                ):
                    rm_b = bpool.tile([128, 4, G4], BF16)
                    for a in range(4):
                        rch = bstage.tile([128, G4], F32, tag="rch")
                        nc.gpsimd.dma_start(
                            rch[:], rmat[ts(a, 128), :])
                        nc.vector.tensor_copy(rm_b[:, a, :], rch[:])

                    nc.gpsimd.memset(hT[:, :, 0, :], 0.0)
                    c_st = bpool.tile([128, 4, B], F32)
                    nc.gpsimd.memset(c_st[:], 0.0)

                    morder = [0, 1, 4, 5, 8, 9, 12, 13, 2, 3, 6, 7, 10, 11, 14, 15]
                    for t in range(n_steps):
                        zp = zpool.tile([128, 16, B], F32)
                        for kk in ((0, 1), (2, 3)):
                            for m in morder:
                                for k in kk:
                                    nc.tensor.matmul(
                                        zp[:, m, :],
                                        rm_b[:, k, ts(PERM[m], 128)],
                                        hT[:, k, t, :],
                                        start=(k == 0), stop=(k == 3))
                        zs = gwork.tile([128, 16, B], F32, tag="zs")
                        for hh in range(2):
                            # gate math for hidden half hh (k-subtiles 2hh, 2hh+1)
                            for g in range(4):
                                m0 = 4 * g + 2 * hh
                                nc.vector.tensor_tensor(
                                    out=zs[:, ds(m0, 2), :], in0=zp[:, ds(m0, 2), :],
                                    in1=preT[:, ds(m0, 2), ds(t * B, B)],
                                    op=ALU.add)
                            gt = gwork.tile([128, 2, B], F32, tag="gt")
                            nc.scalar.activation(
                                gt[:], zs[:, ds(4 * GATE_G + 2 * hh, 2), :], AF.Tanh)
                            hs = gwork.tile([128, 3, 2, B], F32, tag="hs")
                            for gi, g in enumerate((GATE_I, GATE_F, GATE_O)):
                                m0 = 4 * g + 2 * hh
                                nc.vector.tensor_scalar(
                                    out=hs[:, gi], in0=zs[:, ds(m0, 2), :],
                                    scalar1=0.2, scalar2=0.5,
                                    op0=ALU.mult, op1=ALU.add)
                                nc.vector.tensor_scalar(
                                    out=hs[:, gi], in0=hs[:, gi],
                                    scalar1=1.0, scalar2=0.0,
                                    op0=ALU.min, op1=ALU.max)
                            t1 = gwork.tile([128, 2, B], F32, tag="t1")
                            nc.vector.tensor_tensor(out=t1[:], in0=hs[:, 0],
                                                    in1=gt[:], op=ALU.mult)
                            cs = c_st[:, ds(2 * hh, 2), :]
                            nc.vector.tensor_tensor(out=cs, in0=hs[:, 1], in1=cs,
                                                    op=ALU.mult)
                            nc.vector.tensor_tensor(out=cs, in0=cs, in1=t1[:],
                                                    op=ALU.add)
                            tct = gwork.tile([128, 2, B], F32, tag="tct")
                            nc.scalar.activation(tct[:], cs, AF.Tanh)
                            nc.vector.tensor_tensor(
                                out=hT[:, ds(2 * hh, 2), t + 1, :],
                                in0=hs[:, 2], in1=tct[:], op=ALU.mult)

                # ---- Phase C: broadcast h via masked AllReduce ----
                with tc.tile_pool(name="cpool", bufs=1) as cpool:
                    contrib = cpool.tile([128, 8, T, B], BF16)
                    nc.vector.tensor_scalar(
                        out=contrib[:, 0:4], in0=hT[:, :, 1:T + 1, :],
                        scalar1=mf_t[:], scalar2=None, op0=ALU.mult)
                    nc.vector.tensor_scalar(
                        out=contrib[:, 4:8], in0=hT[:, :, T:0:-1, :],
                        scalar1=mb_t[:], scalar2=None, op0=ALU.mult)
                    cin = dram.tile([128, 8, T, B], BF16, tag="cin")
                    cout = dram.tile([128, 8, T, B], BF16, tag="cout")
                    nc.gpsimd.dma_start(cin[:], contrib[:])
                    nc.gpsimd.collective_compute(
                        "AllReduce", ALU.add,
                        replica_groups=[list(range(NCORES))],
                        ins=[cin.opt()], outs=[cout.opt()])
                    hTa = persist.tile([128, 8, T, B], BF16, tag="hTa")
                    nc.gpsimd.dma_start(hTa[:], cout[:])

            # ---- Phase D: dense + softmax (vocab shard) ----
            ones1 = persist.tile([1, 128], BF16, tag="ones1")
            nc.gpsimd.memset(ones1[:], 1.0)
            bd_sb = persist.tile([1, VC], BF16, tag="bd_sb")
            nc.gpsimd.dma_start(bd_sb[:], bd[:])

            with (
                tc.tile_pool(name="expbig", bufs=1) as expbig,
                tc.tile_pool(name="wdpool", bufs=2) as wdpool,
                tc.tile_pool(name="dps", bufs=4, space="PSUM") as dps,
                tc.tile_pool(name="dwork", bufs=3) as dwork,
            ):
                expv = expbig.tile([128, NTT, NVCH, VCH], FP16)
                sump = persist.tile([128, NTT, NVCH], F32, tag="sump")
                outr = out.rearrange("b t v -> t b v")
                for v in range(NVCH):
                    wdt = wdpool.tile([128, NKD, VCH], BF16, tag="wdt")
                    nc.gpsimd.dma_start(
                        wdt[:],
                        wd.rearrange("(a p) v -> p a v", p=128)[:, :, ts(v, VCH)])
                    for j in range(NTT):
                        rows = 128 if j < NTT - 1 else NTOK - 128 * (NTT - 1)
                        nst = rows // B
                        ps = dps.tile([128, VCH], F32, tag="dping")
                        for k in range(NKD):
                            nc.tensor.matmul(
                                ps[:rows, :],
                                hTa[:, k, ds(j * 8, nst), :],
                                wdt[:, k, :],
                                start=(k == 0), stop=False)
                        nc.tensor.matmul(ps[:rows, :], ones1[:, :rows],
                                         bd_sb[:, ts(v, VCH)],
                                         start=False, stop=True)
                        nc.scalar.activation(
                            expv[:rows, j, v, :], ps[:rows, :], AF.Exp,
                            accum_out=sump[:rows, j, v:v + 1])

                sred = persist.tile([128, NTT, 1], F32, tag="sred")
                nc.vector.tensor_reduce(sred[:], sump[:],
                                        axis=mybir.AxisListType.X, op=ALU.add)
                sin = dram.tile([128, NTT], F32, tag="sin")
                sout = dram.tile([128, NTT], F32, tag="sout")
                nc.gpsimd.dma_start(sin[:], sred[:, :, 0])
                nc.gpsimd.collective_compute(
                    "AllReduce", ALU.add,
                    replica_groups=[list(range(NCORES))],
                    ins=[sin.opt()], outs=[sout.opt()])
                gsum = persist.tile([128, NTT], F32, tag="gsum")
                nc.gpsimd.dma_start(gsum[:], sout[:])
                rcp = persist.tile([128, NTT], F32, tag="rcp")
                nc.vector.reciprocal(rcp[:], gsum[:])

                for j in range(NTT):
                    rows = 128 if j < NTT - 1 else NTOK - 128 * (NTT - 1)
                    for v in range(NVCH):
                        ot = dwork.tile([128, VCH], F32, tag="ot")
                        nc.vector.tensor_scalar(
                            out=ot[:rows, :], in0=expv[:rows, j, v, :],
                            scalar1=rcp[:rows, j:j + 1], scalar2=None, op0=ALU.mult)
                        nc.gpsimd.dma_start(
                            outr[ds(j * 8, rows // B), :, ts(v, VCH)], ot[:rows, :])

    nc.compile()
    return nc


def _prep_inputs(tokens, emb, k_fwd, r_fwd, b_fwd, k_bwd, r_bwd, b_bwd, Wd, bd):
    tokens = np.asarray(tokens)
    tok_f = np.ascontiguousarray(tokens.T.reshape(-1)).astype(np.int32)
    tok_b = np.ascontiguousarray(tokens[:, ::-1].T.reshape(-1)).astype(np.int32)
    emb = np.asarray(emb, np.float32)
    wd_bf = np.asarray(Wd, np.float32).astype(ml_dtypes.bfloat16)
    bd_bf = np.asarray(bd, np.float32).astype(ml_dtypes.bfloat16)[None, :]
    in_maps = []
    for c in range(NCORES):
        is_b = (c == 1)
        in_maps.append({
            "tok": tok_b if is_b else tok_f,
            "emb": emb,
            "kmat": np.asarray(k_bwd if is_b else k_fwd, np.float32),
            "rmat": np.asarray(r_bwd if is_b else r_fwd, np.float32),
            "bvec": np.asarray(b_bwd if is_b else b_fwd, np.float32),
            "wd": np.ascontiguousarray(wd_bf[:, c * VC:(c + 1) * VC]),
            "bd": np.ascontiguousarray(bd_bf[:, c * VC:(c + 1) * VC]),
            "maskf": np.full((128, 1), 1.0 if c == 0 else 0.0, np.float32),
            "maskb": np.full((128, 1), 1.0 if c == 1 else 0.0, np.float32),
        })
    return in_maps


def kernel(**inputs) -> np.ndarray:
    global _BUILT
    if _BUILT is None:
        _BUILT = build_kernel()
    in_maps = _prep_inputs(**inputs)
    res = run_bass_kernel_spmd(_BUILT, in_maps, core_ids=list(range(NCORES)))
    return np.concatenate([res.results[c]["out"] for c in range(NCORES)], axis=2)

